# revision 29
# baseline (speedup 1.0000x reference)
"""MiniMax sparse-MoE block on 8 Trainium2 NeuronCores.

Strategy (expert-parallel, per the sharding hint):
  - Router (gates matmul + sigmoid + top-2 + weight normalization) runs on
    host CPU with exactly the reference's jax ops, bit-matching its
    routing decisions.  This *is* the dispatch step: tokens are gathered
    per selected expert ("all-to-all by top-k expert index") while
    building the per-core input shards.
  - Each of the 8 cores owns E/8 = 2 experts.  A core runs the SwitchGLU
    MLP (silu(x@w_gate) * (x@w_up)) @ w_down for the tokens routed to its
    experts only (capacity = max expert load, rounded up), with weights
    stationary on the PE array and tokens as the moving operand
    (activations kept transposed: [H, tokens]).
  - Matmuls run in fp16 (half the HBM traffic of fp32, full-rate PE);
    PSUM accumulation is fp32.  MOE_MM_MODE=f32r/f32 selects higher
    precision modes.
  - Host combines: y[t] = sum over the token's 2 experts of
    sel_weight * expert_out — two fp32 terms, order-independent.
"""

import os
import sys
import functools

for _p in ("/opt/trn_rl_repo", "/root/.axon_site/_ro/trn_rl_repo"):
    if os.path.isdir(_p) and _p not in sys.path:
        sys.path.append(_p)

import numpy as np

T, H, F, E, KTOP = 2048, 1024, 1024, 16, 2
NCORES = 8
EPC = E // NCORES  # experts per core
P = 128
KO = H // P  # contraction chunks per 1024-dim
FB = F // P  # 128-blocks of F
HB = H // P  # 128-blocks of H

# "f16"  = fp16 operands (half the weight DMA bytes, full-rate PE,
#          ~2e-4 rel err per matmul from operand quantization)
# "f32r" = float32r single-pass PE mode (~1.5e-4 rel err per matmul)
# "f32"  = exact fp32 PE mode (4x slower)
MM_MODE = os.environ.get("MOE_MM_MODE", "f16")
WP_BUFS = int(os.environ.get("MOE_WP_BUFS", "16"))
CAP_ALIGN = int(os.environ.get("MOE_CAP_ALIGN", "64"))
WFB = int(os.environ.get("MOE_WFB", "2"))  # f-blocks per weight DMA
YSPLIT = os.environ.get("MOE_YSPLIT", "0") == "1"
FSPLIT = int(os.environ.get("MOE_FSPLIT", "6"))  # f-blocks in the early y burst
YPRE = int(os.environ.get("MOE_YPRE", "6"))  # y chains pre-started

LAST_RESULTS = None  # BassKernelResults of the most recent device run


def _chunks(cap):
    """Split cap into moving-dim chunks <= 512 (PSUM bank / fp32 AP limit)."""
    out, rem, n = [], cap, -(-cap // 512)
    for i in range(n):
        c = min(512, rem, -(-rem // ((n - i) * 64)) * 64)
        out.append(c)
        rem -= c
    assert sum(out) == cap and all(0 < c <= 512 for c in out), (cap, out)
    return out


@functools.lru_cache(maxsize=4)
def _build_program(caps):
    import concourse.mybir as mybir
    import concourse.tile as tile
    from concourse import bacc

    f32 = mybir.dt.float32
    mm_dt = {"f16": mybir.dt.float16,
             "f32r": mybir.dt.float32r,
             "f32": f32}[MM_MODE]
    silu = mybir.ActivationFunctionType.Silu

    nc = bacc.Bacc("TRN2", target_bir_lowering=False, debug=False,
                   num_devices=NCORES)
    tc_kwargs = {}
    if os.environ.get("MOE_POOL_QUEUE", "0") == "1":
        tc_kwargs["pool_alloc_mode"] = "queue"

    xt_d, wg_d, wu_d, wd_d, yt_d = [], [], [], [], []
    for s in range(EPC):
        cap = caps[s]
        xt_d.append(nc.dram_tensor(f"xt{s}", [P, KO, cap], mm_dt,
                                   kind="ExternalInput").ap())
        wg_d.append(nc.dram_tensor(f"wg{s}", [P, FB, H], mm_dt,
                                   kind="ExternalInput").ap())
        wu_d.append(nc.dram_tensor(f"wu{s}", [P, FB, H], mm_dt,
                                   kind="ExternalInput").ap())
        wd_d.append(nc.dram_tensor(f"wd{s}", [P, HB, F], mm_dt,
                                   kind="ExternalInput").ap())
        yt_d.append(nc.dram_tensor(f"yt{s}", [HB, P, cap], f32,
                                   kind="ExternalOutput").ap())

    def mm(ps, lhsT, rhs, start, stop):
        nc.tensor.matmul(ps, lhsT=lhsT, rhs=rhs, start=start, stop=stop)

    with tile.TileContext(nc, **tc_kwargs) as tc:
        with (
            tc.tile_pool(name="xp", bufs=2) as xp,
            tc.tile_pool(name="wp", bufs=WP_BUFS) as wp,
            tc.tile_pool(name="sp", bufs=6) as sp,
            tc.tile_pool(name="hp", bufs=2) as hp,
            tc.tile_pool(name="op", bufs=6) as op,
            tc.tile_pool(name="pp", bufs=8, space="PSUM") as pp,
        ):
            capmax = max(caps)
            for s in range(EPC):
                cap = caps[s]
                cols = _chunks(cap)
                col_off = [0]
                for c in cols:
                    col_off.append(col_off[-1] + c)
                xt = xp.tile([P, KO, capmax], mm_dt, tag="xt", name=f"xt{s}")
                xt = xt[:, :, :cap]
                nc.sync.dma_start(xt, xt_d[s])
                h_sb = hp.tile([P, FB, capmax], mm_dt, tag="h", name=f"h{s}")
                h_sb = h_sb[:, :, :cap]
                ysplit = YSPLIT and len(cols) == 1
                psy_live, wd_t = [], []
                for f0 in range(0, FB, WFB):
                    wgf2 = wp.tile([P, WFB, KO, P], mm_dt, tag="w", name="wgf2")
                    nc.sync.dma_start(
                        wgf2, wg_d[s][:, f0:f0 + WFB].rearrange(
                            "p f (ko m) -> p f ko m", m=P))
                    wuf2 = wp.tile([P, WFB, KO, P], mm_dt, tag="w", name="wuf2")
                    nc.sync.dma_start(
                        wuf2, wu_d[s][:, f0:f0 + WFB].rearrange(
                            "p f (ko m) -> p f ko m", m=P))
                    for fj in range(WFB):
                        f = f0 + fj
                        wgf = wgf2[:, fj]
                        wuf = wuf2[:, fj]
                        for ci, ncol in enumerate(cols):
                            c0, c1 = col_off[ci], col_off[ci + 1]
                            psg = pp.tile([P, ncol], f32, tag="ps")
                            psu = pp.tile([P, ncol], f32, tag="ps")
                            for k in range(KO):
                                mm(psg, wgf[:, k], xt[:, k, c0:c1], k == 0, k == KO - 1)
                            for k in range(KO):
                                mm(psu, wuf[:, k], xt[:, k, c0:c1], k == 0, k == KO - 1)
                            sg = sp.tile([P, ncol], f32, tag="sg")
                            nc.scalar.activation(sg, psg, silu)
                            nc.vector.tensor_mul(out=h_sb[:, f, c0:c1], in0=sg, in1=psu)
                        if ysplit and f == FSPLIT - 1:
                            # pre-start YPRE y chains on the first FSPLIT
                            # f-blocks while g/u of the rest proceeds
                            ncol = cols[0]
                            for hb0 in range(0, YPRE, WFB):
                                wdf2 = wp.tile([P, WFB, FB, P], mm_dt,
                                               tag="w", name="wdf2")
                                nc.sync.dma_start(
                                    wdf2, wd_d[s][:, hb0:hb0 + WFB].rearrange(
                                        "p h (fb m) -> p h fb m", m=P))
                                wd_t.append(wdf2)
                            for hb in range(YPRE):
                                psy = pp.tile([P, ncol], f32, tag="ps",
                                              name=f"psy{hb}")
                                for f2 in range(FSPLIT):
                                    mm(psy, wd_t[hb // WFB][:, hb % WFB, f2],
                                       h_sb[:, f2], f2 == 0, False)
                                psy_live.append(psy)
                if ysplit:
                    ncol = cols[0]
                    for hb in range(YPRE):
                        psy = psy_live[hb]
                        for f2 in range(FSPLIT, FB):
                            mm(psy, wd_t[hb // WFB][:, hb % WFB, f2],
                               h_sb[:, f2], False, f2 == FB - 1)
                        ysb = op.tile([P, ncol], f32, tag="y")
                        nc.vector.tensor_copy(out=ysb, in_=psy)
                        nc.scalar.dma_start(yt_d[s][hb], ysb)
                    rest0 = YPRE
                else:
                    rest0 = 0
                for hb0 in range(rest0, HB, WFB):
                    wdf2 = wp.tile([P, WFB, FB, P], mm_dt, tag="w", name="wdf2")
                    nc.sync.dma_start(
                        wdf2, wd_d[s][:, hb0:hb0 + WFB].rearrange(
                            "p h (fb m) -> p h fb m", m=P))
                    for hj in range(WFB):
                        hb = hb0 + hj
                        wdf = wdf2[:, hj]
                        for ci, ncol in enumerate(cols):
                            c0, c1 = col_off[ci], col_off[ci + 1]
                            psy = pp.tile([P, ncol], f32, tag="ps")
                            for f in range(FB):
                                mm(psy, wdf[:, f], h_sb[:, f, c0:c1], f == 0, f == FB - 1)
                            ysb = op.tile([P, ncol], f32, tag="y")
                            nc.vector.tensor_copy(out=ysb, in_=psy)
                            nc.scalar.dma_start(yt_d[s][hb, :, c0:c1], ysb)

    nc.compile()
    return nc


def _route_np(x, gate_w, bias):
    """Numpy fallback router (same math, host BLAS numerics)."""
    gates = x.astype(np.float32) @ gate_w.T
    orig = 1.0 / (1.0 + np.exp(-gates))
    corrected = orig + bias
    inds = np.argsort(-corrected, axis=-1, kind="stable")[:, :KTOP].astype(np.int32)
    sel = np.take_along_axis(orig, inds, axis=-1)
    sel = sel / (sel.sum(axis=-1, keepdims=True) + 1e-20)
    return inds, sel.astype(np.float32)


def _route(x, gate_w, bias):
    """Top-2 routing with exactly the reference's jax ops on CPU."""
    try:
        import jax
        import jax.numpy as jnp
        cpu = jax.devices("cpu")[0]
    except Exception:
        return _route_np(x, gate_w, bias)
    with jax.default_device(cpu):
        xd = jax.device_put(x, cpu)
        gd = jax.device_put(gate_w, cpu)
        bd = jax.device_put(bias, cpu)
        gates = jnp.einsum("th,eh->te", xd.astype(jnp.float32), gd)
        orig = jax.nn.sigmoid(gates)
        corrected = orig + bd
        _, inds = jax.lax.top_k(corrected, KTOP)
        sel = jnp.take_along_axis(orig, inds, axis=-1)
        sel = sel / (jnp.sum(sel, axis=-1, keepdims=True) + 1e-20)
        sel = sel.astype(x.dtype)
    return np.asarray(inds), np.asarray(sel)


_PACK_CACHE = {}


NP_MM_DT = np.float16 if MM_MODE == "f16" else np.float32


def _pack(w):
    """[1024, 1024] -> [128, 8, 1024]: out[p, b, k*128+m] = w[k*128+p, b*128+m].

    Partition-major so a [p, f0:f1] DMA slice is one contiguous multi-KB
    run per partition (big DMA descriptors)."""
    return np.ascontiguousarray(
        w.reshape(8, P, 8, P).transpose(1, 2, 0, 3).reshape(P, 8, 8 * P)
        .astype(NP_MM_DT))


def kernel(x, gate_w, w_gate, w_up, w_down, e_score_correction_bias):
    global LAST_RESULTS
    from concourse import bass_utils

    x = np.asarray(x, dtype=np.float32)
    inds, sel = _route(x, np.asarray(gate_w, np.float32),
                       np.asarray(e_score_correction_bias, np.float32))

    # dispatch: token lists per expert
    tok_idx, tok_w = [], []
    for e in range(E):
        rows, slots = np.nonzero(inds == e)
        tok_idx.append(rows)
        tok_w.append(sel[rows, slots])
    counts = np.array([len(t) for t in tok_idx])

    # Pair heavy experts with light ones: slot 0 of each core gets one of
    # the 8 largest experts, slot 1 one of the 8 smallest, so slot 1's
    # capacity (max over its experts) can be smaller than slot 0's.
    order = np.argsort(-counts, kind="stable")
    assign = [(int(order[c]), int(order[E - 1 - c])) for c in range(NCORES)]

    def _cap(n):
        if MM_MODE == "f16":
            return max(64, -(-max(n, 1) // CAP_ALIGN) * CAP_ALIGN)
        return max(256, -(-max(n, 1) // 64) * 64)

    caps = tuple(_cap(int(counts[[assign[c][s] for c in range(NCORES)]].max()))
                 for s in range(EPC))

    nc = _build_program(caps)

    # weight packing (cached on the weight buffers' identity)
    wkey = (id(w_gate), id(w_up), id(w_down),
            w_gate.shape if hasattr(w_gate, "shape") else None)
    packed = _PACK_CACHE.get(wkey)
    if packed is None:
        wg = np.asarray(w_gate, np.float32)
        wu = np.asarray(w_up, np.float32)
        wd = np.asarray(w_down, np.float32)
        packed = ([_pack(wg[e]) for e in range(E)],
                  [_pack(wu[e]) for e in range(E)],
                  [_pack(wd[e]) for e in range(E)])
        _PACK_CACHE.clear()
        _PACK_CACHE[wkey] = packed
    wg_p, wu_p, wd_p = packed

    in_maps = []
    for c in range(NCORES):
        m = {}
        for s in range(EPC):
            e = assign[c][s]
            xt = np.zeros((P, KO, caps[s]), NP_MM_DT)
            cnt = len(tok_idx[e])
            if cnt:
                g = x[tok_idx[e]].astype(NP_MM_DT)  # [cnt, H]
                xt[:, :, :cnt] = g.reshape(cnt, KO, P).transpose(2, 1, 0)
            m[f"xt{s}"] = xt
            m[f"wg{s}"] = wg_p[e]
            m[f"wu{s}"] = wu_p[e]
            m[f"wd{s}"] = wd_p[e]
        in_maps.append(m)

    res = None
    last_err = None
    for attempt in range(3):
        try:
            res = bass_utils.run_bass_kernel_spmd(
                nc, in_maps, core_ids=list(range(NCORES)))
            break
        except Exception as err:  # transient NRT/device errors happen
            last_err = err
            import time as _time
            _time.sleep(3.0 * (attempt + 1))
    if res is None:
        raise last_err
    LAST_RESULTS = res

    y = np.zeros((x.shape[0], H), np.float32)
    for c in range(NCORES):
        for s in range(EPC):
            e = assign[c][s]
            cnt = len(tok_idx[e])
            if not cnt:
                continue
            yt = res.results[c][f"yt{s}"].reshape(H, caps[s])
            y[tok_idx[e]] += tok_w[e][:, None] * yt[:, :cnt].T
    return y

